# revision 19
# baseline (speedup 1.0000x reference)
"""AttentionBlock (GroupNorm + single-head self-attention + residual) on 8 TRN2 cores.

Data-parallel over batch: 32 samples -> 4 per core; weights replicated.

Algebraic folds (softmax over keys m kills terms constant in m):
  scores'[n,m] = (A^T h[n] + c)^T h[m],  A = Wq^T Wk * C^-0.5, c = Wk^T bq * C^-0.5
  -> ONE projection u = A^T xn + c replaces both Q and K.
  attn @ V @ Wp^T = attn @ (h (Wp Wv)^T) + Wp bv
  -> ONE projection vp = Btw^T xn (Btw = (Wp Wv)^T) replaces V and out-proj;
     Wp bv joins bp in the residual shift.

Per-core program (per sample, N=1024 tokens, C=512 channels):
  xn_mm [c, n] fp8   = x*scale + shift        (matmul operand)
  xn_res[c, n] bf16  = x*scale + (shift + bp_eff)   (residual, bias pre-folded)
  u   [d', n] fp8    = A-slices.T @ xn_mm + c       (lhsT = host A)
  vp  [m, e]  fp8    = xn-slices.T @ Btw            (keys on partitions)
  eT  [m, n]  fp8    = exp(xn-slices.T @ u)         (scale folded into A)
  rs  [1, n]         = ones2.T @ eT                 (DoubleRow ones matmul)
  rb  [128, n] f32   = 1/rs broadcast via K=1 bf16 matmul + reciprocal
  out [e, n] bf16    = (vp-slices.T @ eT) * rb + xn_res  -> DMA out

All matmuls fp8_e4m3 with MatmulPerfMode.DoubleRow (2 k-subtiles of 128
packed per instruction, 0.5 cycles/row = 2x bf16 rate). fp32 PSUM.
x is host-cast to bf16 (halves input DMA), out returned bf16 and upcast
on host. Emulated end-to-end rel err ~7e-3 vs 2e-2 budget.

GroupNorm stats are just-in-time: x[s+2] DMA issued at sample s's top,
stats+affine for s+1 run mid-sample-s (hidden under attention).
"""

import sys

if "/opt/trn_rl_repo" not in sys.path:
    sys.path.insert(0, "/opt/trn_rl_repo")

from contextlib import ExitStack

import numpy as np

import concourse.bass as bass
import concourse.tile as tile
from concourse import bacc, mybir
from concourse.bass_utils import run_bass_kernel_spmd

N_CORES = 8
B, C, H, W = 32, 512, 32, 32
HW = H * W            # tokens per sample (N)
SPC = B // N_CORES    # samples per core
G = 8                 # groups
GSZ = C // G          # channels per group (64)
EPS = 1e-5
P = 128               # partitions
CT = C // P           # channel subtiles (4)
NT = HW // P          # token subtiles (8)
SCALE = C ** -0.5
FREE = 512            # PSUM one-bank free width (fp32)
NJC = HW // FREE      # 2

F32 = mybir.dt.float32
BF16 = mybir.dt.bfloat16
FP8 = mybir.dt.float8e4
AF = mybir.ActivationFunctionType
ALU = mybir.AluOpType
DR = mybir.MatmulPerfMode.DoubleRow

QUANT = "fp8"         # "fp8" (DoubleRow) or "bf16" (plain matmuls)
MMDT = FP8 if QUANT == "fp8" else BF16


def _declare_io(nc):
    def inp(name, shape, dt):
        return nc.dram_tensor(name, list(shape), dt, kind="ExternalInput").ap()

    aps = {
        "x": inp("x", (SPC, C, HW), BF16),
        "at": inp("at", (C, C), MMDT),           # A[c, d'] (scale folded in)
        "btw": inp("btw", (C, C), MMDT),         # Btw[c, e] = (Wp Wv)^T
        "smalls": inp("smalls", (P, 19), F32),
        "member_t": inp("member_t", (2, P), F32),
        "ones2": inp("ones2", (P, NT, 16), MMDT),  # 16-wide: dual-fp8 ldweights needs 16B-aligned k-pair stride
        "ones_row": inp("ones_row", (1, P), BF16),
        "out": nc.dram_tensor("out", [SPC, C, HW], BF16, kind="ExternalOutput").ap(),
    }
    return aps


def _build_tile_kernel(ctx: ExitStack, tc: tile.TileContext, aps):
    nc = tc.nc

    singles = ctx.enter_context(tc.tile_pool(name="singles", bufs=1))
    wpool = ctx.enter_context(tc.tile_pool(name="wpool", bufs=1))
    xall = ctx.enter_context(tc.tile_pool(name="xall", bufs=1))
    xnpool = ctx.enter_context(tc.tile_pool(name="xnpool", bufs=2))
    upool = ctx.enter_context(tc.tile_pool(name="upool", bufs=2))
    vpool = ctx.enter_context(tc.tile_pool(name="vpool", bufs=2))
    epool = ctx.enter_context(tc.tile_pool(name="epool", bufs=2))
    rpool = ctx.enter_context(tc.tile_pool(name="rpool", bufs=2))
    respool = ctx.enter_context(tc.tile_pool(name="respool", bufs=3))
    stat = ctx.enter_context(tc.tile_pool(name="stat", bufs=2))
    sall = ctx.enter_context(tc.tile_pool(name="sall", bufs=1))

    psum_big = ctx.enter_context(tc.tile_pool(name="psum_big", bufs=3, space="PSUM"))
    psum_sm = ctx.enter_context(tc.tile_pool(name="psum_sm", bufs=2, space="PSUM"))

    # ---- small constants ----
    smalls = singles.tile([P, 19], F32, tag="smalls")
    nc.sync.dma_start(out=smalls[:], in_=aps["smalls"][:])
    cvec_c = smalls[:, 0:CT]
    bp_c = smalls[:, CT:2 * CT]
    gamma_c = smalls[:, 2 * CT:3 * CT]
    beta_c = smalls[:, 3 * CT:4 * CT]
    member = smalls[:, 16:18]
    member_t = singles.tile([2, P], F32, tag="member_t")
    nc.sync.dma_start(out=member_t[:], in_=aps["member_t"][:])
    ones2 = singles.tile([P, NT, 16], MMDT, tag="ones2")
    nc.sync.dma_start(out=ones2[:], in_=aps["ones2"][:])
    ones_row = singles.tile([1, P], BF16, tag="ones_row")
    nc.sync.dma_start(out=ones_row[:], in_=aps["ones_row"][:])

    I32 = mybir.dt.int32
    magic = singles.tile([2, SPC * CT], I32, tag="magic")
    nc.vector.memset(magic[:], 0x5F3759DF)

    # prime the ACT Exp spline table before any real dependency needs it
    warm = singles.tile([1, 1], F32, tag="warm")
    nc.vector.memset(warm[:], 1.0)
    nc.scalar.activation(out=warm[:], in_=warm[:], func=AF.Exp)

    # ---- weights resident in SBUF: [128, CT(k-subtile), C] ----
    at_t = wpool.tile([P, CT, C], MMDT, tag="at")
    btw_t = wpool.tile([P, CT, C], MMDT, tag="btw")
    wdmas = []
    for ct in range(CT):
        wdmas.append(nc.gpsimd.dma_start(out=at_t[:, ct, :],
                                         in_=aps["at"][ct * P:(ct + 1) * P, :]))
        wdmas.append(nc.gpsimd.dma_start(out=btw_t[:, ct, :],
                                         in_=aps["btw"][ct * P:(ct + 1) * P, :]))

    # ---- DoubleRow matmul chain helper ----
    # lhsT, rhs: [128, nk, *] SBUF tiles; msl/nsl slices of the last dim.
    def mm_chain(ps, lhsT, msl, rhs, nsl, nk):
        if QUANT == "fp8":
            for i, k in enumerate(range(0, nk, 2)):
                nc.tensor.matmul(ps, lhsT[:, k:k + 2, msl], rhs[:, k:k + 2, nsl],
                                 start=(i == 0), stop=(k + 2 >= nk), perf_mode=DR)
        else:
            for k in range(nk):
                nc.tensor.matmul(ps, lhsT[:, k, msl], rhs[:, k, nsl],
                                 start=(k == 0), stop=(k == nk - 1))

    # ======== GroupNorm statistics (per-sample, just-in-time) ========
    # stats_all[j, s, t, k]: group-local j, sample s, channel-tile t,
    # k=0 mean / k=1 E[x^2]; sc/sh/shp: per-channel scale/shift columns.
    stats_all = sall.tile([2, SPC, CT, 2], F32, tag="stats_all")
    sc = sall.tile([P, SPC, CT], F32, tag="sc")
    sh = sall.tile([P, SPC, CT], F32, tag="sh")
    shp = sall.tile([P, SPC, CT], F32, tag="shp")

    xr = {}  # (s, ct) -> resident x tile (bf16)

    def x_dma(s, split=False):
        for ct in range(CT):
            xt = xall.tile([P, HW], BF16, tag=f"x{s % 3}_{ct}")
            if split:
                nc.sync.dma_start(out=xt[:, 0:512],
                                  in_=aps["x"][s, ct * P:(ct + 1) * P, 0:512])
                nc.sync.dma_start(out=xt[:, 512:1024],
                                  in_=aps["x"][s, ct * P:(ct + 1) * P, 512:1024])
            else:
                nc.sync.dma_start(out=xt[:], in_=aps["x"][s, ct * P:(ct + 1) * P, :])
            xr[(s, ct)] = xt

    def gn_stats(s):
        partials = stat.tile([P, CT, 2], F32, tag="partials")
        for ct in range(CT):
            xt = xr[(s, ct)]
            st6 = stat.tile([P, 2, 6], F32, tag="st6")
            nc.vector.bn_stats(out=st6[:, 0, :], in_=xt[:, 0:512])
            nc.vector.bn_stats(out=st6[:, 1, :], in_=xt[:, 512:1024])
            nc.vector.bn_aggr(out=partials[:, ct, :], in_=st6[:])
            nc.vector.scalar_tensor_tensor(
                out=partials[:, ct, 1:2], in0=partials[:, ct, 0:1],
                scalar=partials[:, ct, 0:1], in1=partials[:, ct, 1:2],
                op0=ALU.mult, op1=ALU.add)
        ps = psum_sm.tile([2, CT * 2], F32, tag="sm")
        nc.tensor.matmul(ps[:], member[:],
                         partials[:].rearrange("p t j -> p (t j)"),
                         start=True, stop=True)
        nc.vector.tensor_copy(out=stats_all[:, s, :, :],
                              in_=ps[:].rearrange("p (t j) -> p t j", j=2))

    def gn_affine(s0, ns):
        # rstd = rsqrt(var+eps) via bit-trick + 3 Newton iters (DVE only)
        mv = stats_all[:, s0:s0 + ns, :, 0]
        sv = stats_all[:, s0:s0 + ns, :, 1]
        msq = stat.tile([2, ns, CT], F32, tag="msq")
        nc.vector.tensor_mul(out=msq[:], in0=mv, in1=mv)
        nc.vector.tensor_sub(out=sv, in0=sv, in1=msq[:])
        vadd = stat.tile([2, ns, CT], F32, tag="vadd")
        nc.vector.tensor_scalar_add(out=vadd[:], in0=sv, scalar1=float(EPS))
        z = stat.tile([2, ns, CT], F32, tag="z")
        z_i = z[:].bitcast(I32)
        nc.vector.tensor_scalar(out=z_i, in0=vadd[:].bitcast(I32), scalar1=1,
                                scalar2=None, op0=ALU.arith_shift_right)
        mg = magic[:, 0:ns * CT].rearrange("p (s t) -> p s t", t=CT)
        nc.vector.scalar_tensor_tensor(out=z_i, in0=mg, scalar=0, in1=z_i,
                                       op0=ALU.bypass, op1=ALU.subtract)
        nt_ = stat.tile([2, ns, CT], F32, tag="nt")
        for _ in range(3):
            nc.vector.tensor_mul(out=nt_[:], in0=z[:], in1=z[:])
            nc.vector.tensor_mul(out=nt_[:], in0=nt_[:], in1=vadd[:])
            nc.vector.tensor_scalar(out=nt_[:], in0=nt_[:], scalar1=-0.5,
                                    scalar2=1.5, op0=ALU.mult, op1=ALU.add)
            nc.vector.tensor_mul(out=z[:], in0=z[:], in1=nt_[:])
        nc.vector.tensor_copy(out=sv, in_=z[:])
        ab = stat.tile([2, ns, CT, 2], F32, tag="ab")
        nc.vector.tensor_copy(out=ab[:, :, :, 0], in_=sv)
        nc.vector.scalar_tensor_tensor(out=ab[:, :, :, 1], in0=mv, scalar=-1.0,
                                       in1=sv, op0=ALU.mult, op1=ALU.mult)
        sb_ps = psum_sm.tile([P, ns * CT * 2], F32, tag="sm")
        nc.tensor.matmul(sb_ps[:], member_t[:],
                         ab[:].rearrange("p s t j -> p (s t j)"),
                         start=True, stop=True)
        sb = stat.tile([P, ns, CT, 2], F32, tag="sb")
        nc.vector.tensor_copy(
            out=sb[:], in_=sb_ps[:].rearrange("p (s t j) -> p s t j", t=CT, j=2))
        for i in range(ns):
            s = s0 + i
            for ct in range(CT):
                nc.vector.tensor_scalar_mul(out=sc[:, s, ct:ct + 1],
                                            in0=gamma_c[:, ct:ct + 1],
                                            scalar1=sb[:, i, ct, 0:1])
                nc.vector.scalar_tensor_tensor(out=sh[:, s, ct:ct + 1],
                                               in0=gamma_c[:, ct:ct + 1],
                                               scalar=sb[:, i, ct, 1:2],
                                               in1=beta_c[:, ct:ct + 1],
                                               op0=ALU.mult, op1=ALU.add)
            nc.vector.tensor_add(out=shp[:, s, :], in0=sh[:, s, :], in1=bp_c)

    # ======== phase 0: sample 0 stats ========
    x_dma(0, split=True)
    gn_stats(0)
    gn_affine(0, 1)

    # ======== main per-sample loop (front software-pipelined) ========
    def front(s):
        xn_mm = xnpool.tile([P, CT, HW], MMDT, tag="xnmm", name=f"xnmm{s}")
        xn_res = xnpool.tile([P, CT, HW], BF16, tag="xnres", name=f"xnres{s}")
        for ct in range(CT):
            nc.vector.tensor_scalar(out=xn_mm[:, ct, :], in0=xr[(s, ct)][:],
                                    scalar1=sc[:, s, ct:ct + 1],
                                    scalar2=sh[:, s, ct:ct + 1],
                                    op0=ALU.mult, op1=ALU.add)
            nc.gpsimd.tensor_scalar(out=xn_res[:, ct, :], in0=xr[(s, ct)][:],
                                    scalar1=sc[:, s, ct:ct + 1],
                                    scalar2=shp[:, s, ct:ct + 1],
                                    op0=ALU.mult, op1=ALU.add)
        u = upool.tile([P, CT, HW], MMDT, tag="u", name=f"u{s}")
        for dt in range(CT):
            ps = psum_big.tile([P, HW], F32, tag="big", name=f"psu{s}_{dt}")
            for jc in range(NJC):
                mm_chain(ps[:, jc * FREE:(jc + 1) * FREE],
                         at_t, slice(dt * P, (dt + 1) * P),
                         xn_mm, slice(jc * FREE, (jc + 1) * FREE), CT)
            nc.vector.tensor_scalar_add(out=u[:, dt, :], in0=ps[:],
                                        scalar1=cvec_c[:, dt:dt + 1])
        return xn_mm, xn_res, u

    fr = front(0)
    for s in range(SPC):
        if s + 2 < SPC:
            x_dma(s + 2)
        if s == 0 and SPC > 1:
            x_dma(1)
        xn_mm, xn_res, u = fr

        # ---- vp[m, e] and eT[m, n] interleaved per token tile ----
        vp = vpool.tile([P, NT, FREE], MMDT, tag="vp")
        eT = epool.tile([P, NT, HW], MMDT, tag="e")
        for nt in range(NT):
            psv = psum_sm.tile([P, FREE], F32, tag="sm")
            mm_chain(psv[:], xn_mm, slice(nt * P, (nt + 1) * P),
                     btw_t, slice(0, C), CT)
            nc.vector.tensor_copy(out=vp[:, nt, :], in_=psv[:])
            pse = psum_big.tile([P, HW], F32, tag="big")
            for jc in range(NJC):
                mm_chain(pse[:, jc * FREE:(jc + 1) * FREE],
                         xn_mm, slice(nt * P, (nt + 1) * P),
                         u, slice(jc * FREE, (jc + 1) * FREE), CT)
            nc.scalar.activation(out=eT[:, nt, :], in_=pse[:], func=AF.Exp)

        # stats for the next sample hide under this sample's attention
        if s + 1 < SPC:
            gn_stats(s + 1)
            gn_affine(s + 1, 1)

        # ---- softmax denominator -> rb = (1/rowsum) broadcast ----
        rs_sb = rpool.tile([1, HW], BF16, tag="rs")
        for jc in range(NJC):
            rps = psum_sm.tile([1, FREE], F32, tag="sm")
            mm_chain(rps[:], ones2, slice(0, 1),
                     eT, slice(jc * FREE, (jc + 1) * FREE), NT)
            nc.vector.tensor_copy(out=rs_sb[:, jc * FREE:(jc + 1) * FREE],
                                  in_=rps[:])
        rb = rpool.tile([P, HW], F32, tag="rb")
        for jc in range(NJC):
            bps = psum_sm.tile([P, FREE], F32, tag="sm")
            nc.tensor.matmul(bps[:], ones_row[:],
                             rs_sb[:, jc * FREE:(jc + 1) * FREE],
                             start=True, stop=True)
            nc.vector.reciprocal_approx_fast(out=rb[:, jc * FREE:(jc + 1) * FREE],
                                             in_=bps[:])

        # next sample's xn + u fill the PE while rb resolves
        if s + 1 < SPC:
            fr = front(s + 1)

        # ---- out = (vp.T @ eT) * rb + xn_res ----
        for et in range(CT):
            ps = psum_big.tile([P, HW], F32, tag="big")
            for jc in range(NJC):
                mm_chain(ps[:, jc * FREE:(jc + 1) * FREE],
                         vp, slice(et * P, (et + 1) * P),
                         eT, slice(jc * FREE, (jc + 1) * FREE), NT)
            t = respool.tile([P, HW], BF16, tag="t")
            nc.vector.tensor_mul(out=t[:], in0=ps[:], in1=rb[:])
            res = respool.tile([P, HW], BF16, tag="res")
            nc.gpsimd.tensor_add(out=res[:], in0=t[:], in1=xn_res[:, et, :])
            nc.sync.dma_start(out=aps["out"][s, et * P:(et + 1) * P, :], in_=res[:])


def build():
    nc = bacc.Bacc("TRN2", target_bir_lowering=False, debug=False)
    aps = _declare_io(nc)
    with tile.TileContext(nc) as tc:
        with ExitStack() as ctx:
            _build_tile_kernel(ctx, tc, aps)
    nc.compile()
    return nc


_cached_nc = None


def _get_nc():
    global _cached_nc
    if _cached_nc is None:
        _cached_nc = build()
    return _cached_nc


def _host_inputs(gamma, beta, Wq, bq, Wk, bk, Wv, bv, Wp, bp):
    import ml_dtypes
    mmnp = ml_dtypes.float8_e4m3 if QUANT == "fp8" else ml_dtypes.bfloat16
    f64 = lambda a: np.asarray(a, dtype=np.float64)
    wcast = lambda a: np.ascontiguousarray(a.astype(np.float32).astype(mmnp))

    a_s = (f64(Wq).T @ f64(Wk)) * SCALE          # [c, d']
    cvec = (f64(Wk).T @ f64(bq)) * SCALE         # [d']
    btw = (f64(Wp) @ f64(Wv)).T                  # [c, e]
    bp_eff = f64(bp) + f64(Wp) @ f64(bv)

    member_t = np.zeros((2, P), np.float32)
    member_t[0, :GSZ] = 1.0
    member_t[1, GSZ:] = 1.0

    smalls = np.zeros((P, 19), np.float32)
    for i, v in enumerate((cvec, bp_eff, gamma, beta)):
        smalls[:, i * CT:(i + 1) * CT] = (
            np.asarray(v, np.float32).reshape(CT, P).T)
    smalls[:GSZ, 16] = 1.0 / GSZ
    smalls[GSZ:, 17] = 1.0 / GSZ
    smalls[:, 18] = 1.0

    return {
        "at": wcast(a_s),
        "btw": wcast(btw),
        "smalls": smalls,
        "member_t": member_t,
        "ones2": np.ones((P, NT, 16), mmnp),
        "ones_row": np.ones((1, P), ml_dtypes.bfloat16),
    }


def run(inputs, trace=False, **kw):
    """Returns (out [B,C,H,W], BassKernelResults)."""
    import ml_dtypes
    nc = _get_nc()
    x = np.ascontiguousarray(
        np.asarray(inputs["x"], np.float32).reshape(B, C, HW)
        .astype(ml_dtypes.bfloat16))
    common = _host_inputs(**{k: v for k, v in inputs.items() if k != "x"})
    in_maps = [dict(common, x=x[c * SPC:(c + 1) * SPC]) for c in range(N_CORES)]
    res = run_bass_kernel_spmd(nc, in_maps, core_ids=list(range(N_CORES)),
                               trace=trace, **kw)
    out = np.concatenate([res.results[c]["out"] for c in range(N_CORES)], axis=0)
    return out.astype(np.float32).reshape(B, C, H, W), res


def kernel(**inputs):
    out, _ = run(inputs)
    return out


# revision 21
# speedup vs baseline: 1.3434x; 1.3434x over previous
"""AttentionBlock (GroupNorm + single-head self-attention + residual) on 8 TRN2 cores.

Data-parallel over batch: 32 samples -> 4 per core; weights replicated.

Algebraic folds (softmax over keys m kills terms constant in m):
  scores'[n,m] = (A^T h[n] + c)^T h[m],  A = Wq^T Wk * C^-0.5, c = Wk^T bq * C^-0.5
  -> ONE projection u = A^T xn + c replaces both Q and K.
  attn @ V @ Wp^T = attn @ (h (Wp Wv)^T) + Wp bv
  -> ONE projection vp = Btw^T xn (Btw = (Wp Wv)^T) replaces V and out-proj;
     Wp bv joins bp in the residual shift.

Per-core program (per sample, N=1024 tokens, C=512 channels):
  xn_mm [c, n] fp8   = x*scale + shift        (matmul operand)
  xn_res[c, n] bf16  = x*scale + (shift + bp_eff)   (residual, bias pre-folded)
  u   [d', n] fp8    = A-slices.T @ xn_mm + c       (lhsT = host A)
  vp  [m, e]  fp8    = xn-slices.T @ Btw            (keys on partitions)
  eT  [m, n]  fp8    = exp(xn-slices.T @ u)         (scale folded into A)
  rs  [1, n]         = ones2.T @ eT                 (DoubleRow ones matmul)
  rb  [128, n] f32   = 1/rs broadcast via K=1 bf16 matmul + reciprocal
  out [e, n] bf16    = (vp-slices.T @ eT) * rb + xn_res  -> DMA out

All matmuls fp8_e4m3 with MatmulPerfMode.DoubleRow (2 k-subtiles of 128
packed per instruction, 0.5 cycles/row = 2x bf16 rate). fp32 PSUM.
x is host-cast to bf16 (halves input DMA), out returned bf16 and upcast
on host. Emulated end-to-end rel err ~7e-3 vs 2e-2 budget.

GroupNorm stats are just-in-time: x[s+2] DMA issued at sample s's top,
stats+affine for s+1 run mid-sample-s (hidden under attention).
"""

import sys

if "/opt/trn_rl_repo" not in sys.path:
    sys.path.insert(0, "/opt/trn_rl_repo")

from contextlib import ExitStack

import numpy as np

import concourse.bass as bass
import concourse.tile as tile
from concourse import bacc, mybir
from concourse.bass_utils import run_bass_kernel_spmd

N_CORES = 8
B, C, H, W = 32, 512, 32, 32
HW = H * W            # tokens per sample (N)
SPC = B // N_CORES    # samples per core
G = 8                 # groups
GSZ = C // G          # channels per group (64)
EPS = 1e-5
P = 128               # partitions
CT = C // P           # channel subtiles (4)
NT = HW // P          # token subtiles (8)
SCALE = C ** -0.5
FREE = 512            # PSUM one-bank free width (fp32)
NJC = HW // FREE      # 2

F32 = mybir.dt.float32
BF16 = mybir.dt.bfloat16
FP8 = mybir.dt.float8e4
AF = mybir.ActivationFunctionType
ALU = mybir.AluOpType
DR = mybir.MatmulPerfMode.DoubleRow

QUANT = "fp8"         # "fp8" (DoubleRow) or "bf16" (plain matmuls)
MMDT = FP8 if QUANT == "fp8" else BF16


def _declare_io(nc):
    def inp(name, shape, dt):
        return nc.dram_tensor(name, list(shape), dt, kind="ExternalInput").ap()

    aps = {
        "x": inp("x", (SPC, C, HW), BF16),
        "at": inp("at", (C, C), MMDT),           # A[c, d'] (scale folded in)
        "btw": inp("btw", (C, C), MMDT),         # Btw[c, e] = (Wp Wv)^T
        "smalls": inp("smalls", (P, 19), F32),
        "member_t": inp("member_t", (2, P), F32),
        "ones2": inp("ones2", (P, NT, 16), MMDT),  # 16-wide: dual-fp8 ldweights needs 16B-aligned k-pair stride
        "ones_row": inp("ones_row", (1, P), BF16),
        "out": nc.dram_tensor("out", [SPC, C, HW], BF16, kind="ExternalOutput").ap(),
    }
    return aps


def _build_tile_kernel(ctx: ExitStack, tc: tile.TileContext, aps):
    nc = tc.nc

    singles = ctx.enter_context(tc.tile_pool(name="singles", bufs=1))
    wpool = ctx.enter_context(tc.tile_pool(name="wpool", bufs=1))
    xall = ctx.enter_context(tc.tile_pool(name="xall", bufs=1))
    xnpool = ctx.enter_context(tc.tile_pool(name="xnpool", bufs=2))
    upool = ctx.enter_context(tc.tile_pool(name="upool", bufs=2))
    vpool = ctx.enter_context(tc.tile_pool(name="vpool", bufs=2))
    epool = ctx.enter_context(tc.tile_pool(name="epool", bufs=2))
    rpool = ctx.enter_context(tc.tile_pool(name="rpool", bufs=2))
    respool = ctx.enter_context(tc.tile_pool(name="respool", bufs=3))
    stat = ctx.enter_context(tc.tile_pool(name="stat", bufs=2))
    sall = ctx.enter_context(tc.tile_pool(name="sall", bufs=1))

    psum_big = ctx.enter_context(tc.tile_pool(name="psum_big", bufs=3, space="PSUM"))
    psum_sm = ctx.enter_context(tc.tile_pool(name="psum_sm", bufs=2, space="PSUM"))

    # ---- small constants ----
    smalls = singles.tile([P, 19], F32, tag="smalls")
    nc.sync.dma_start(out=smalls[:], in_=aps["smalls"][:])
    cvec_c = smalls[:, 0:CT]
    bp_c = smalls[:, CT:2 * CT]
    gamma_c = smalls[:, 2 * CT:3 * CT]
    beta_c = smalls[:, 3 * CT:4 * CT]
    member = smalls[:, 16:18]
    member_t = singles.tile([2, P], F32, tag="member_t")
    nc.sync.dma_start(out=member_t[:], in_=aps["member_t"][:])
    ones2 = singles.tile([P, NT, 16], MMDT, tag="ones2")
    nc.sync.dma_start(out=ones2[:], in_=aps["ones2"][:])
    ones_row = singles.tile([1, P], BF16, tag="ones_row")
    nc.sync.dma_start(out=ones_row[:], in_=aps["ones_row"][:])

    I32 = mybir.dt.int32
    magic = singles.tile([2, SPC * CT], I32, tag="magic")
    nc.vector.memset(magic[:], 0x5F3759DF)

    # prime the ACT Exp spline table before any real dependency needs it
    warm = singles.tile([1, 1], F32, tag="warm")
    nc.vector.memset(warm[:], 1.0)
    nc.scalar.activation(out=warm[:], in_=warm[:], func=AF.Exp)

    # ---- weights resident in SBUF: [128, CT(k-subtile), C] ----
    at_t = wpool.tile([P, CT, C], MMDT, tag="at")
    btw_t = wpool.tile([P, CT, C], MMDT, tag="btw")
    wdmas = []
    for ct in range(CT):
        wdmas.append(nc.gpsimd.dma_start(out=at_t[:, ct, :],
                                         in_=aps["at"][ct * P:(ct + 1) * P, :]))
        wdmas.append(nc.gpsimd.dma_start(out=btw_t[:, ct, :],
                                         in_=aps["btw"][ct * P:(ct + 1) * P, :]))

    # ---- DoubleRow matmul chain helper ----
    # lhsT, rhs: [128, nk, *] SBUF tiles; msl/nsl slices of the last dim.
    def mm_chain(ps, lhsT, msl, rhs, nsl, nk):
        if QUANT == "fp8":
            for i, k in enumerate(range(0, nk, 2)):
                nc.tensor.matmul(ps, lhsT[:, k:k + 2, msl], rhs[:, k:k + 2, nsl],
                                 start=(i == 0), stop=(k + 2 >= nk), perf_mode=DR)
        else:
            for k in range(nk):
                nc.tensor.matmul(ps, lhsT[:, k, msl], rhs[:, k, nsl],
                                 start=(k == 0), stop=(k == nk - 1))

    # ======== GroupNorm statistics (per-sample, just-in-time) ========
    # stats_all[j, s, t, k]: group-local j, sample s, channel-tile t,
    # k=0 mean / k=1 E[x^2]; sc/sh/shp: per-channel scale/shift columns.
    stats_all = sall.tile([2, SPC, CT, 2], F32, tag="stats_all")
    sc = sall.tile([P, SPC, CT], F32, tag="sc")
    sh = sall.tile([P, SPC, CT], F32, tag="sh")
    shp = sall.tile([P, SPC, CT], F32, tag="shp")

    xr = {}  # (s, ct) -> resident x tile (bf16)

    def x_dma(s, split=False):
        for ct in range(CT):
            xt = xall.tile([P, HW], BF16, tag=f"x{s % 3}_{ct}")
            if split:
                nc.sync.dma_start(out=xt[:, 0:512],
                                  in_=aps["x"][s, ct * P:(ct + 1) * P, 0:512])
                nc.sync.dma_start(out=xt[:, 512:1024],
                                  in_=aps["x"][s, ct * P:(ct + 1) * P, 512:1024])
            else:
                nc.sync.dma_start(out=xt[:], in_=aps["x"][s, ct * P:(ct + 1) * P, :])
            xr[(s, ct)] = xt

    def gn_stats(s):
        partials = stat.tile([P, CT, 2], F32, tag="partials")
        for ct in range(CT):
            xt = xr[(s, ct)]
            st6 = stat.tile([P, 2, 6], F32, tag="st6")
            nc.vector.bn_stats(out=st6[:, 0, :], in_=xt[:, 0:512])
            nc.vector.bn_stats(out=st6[:, 1, :], in_=xt[:, 512:1024])
            nc.vector.bn_aggr(out=partials[:, ct, :], in_=st6[:])
            nc.vector.scalar_tensor_tensor(
                out=partials[:, ct, 1:2], in0=partials[:, ct, 0:1],
                scalar=partials[:, ct, 0:1], in1=partials[:, ct, 1:2],
                op0=ALU.mult, op1=ALU.add)
        ps = psum_sm.tile([2, CT * 2], F32, tag="sm")
        nc.tensor.matmul(ps[:], member[:],
                         partials[:].rearrange("p t j -> p (t j)"),
                         start=True, stop=True)
        nc.vector.tensor_copy(out=stats_all[:, s, :, :],
                              in_=ps[:].rearrange("p (t j) -> p t j", j=2))

    def gn_affine(s0, ns):
        # rstd = rsqrt(var+eps) via bit-trick + 3 Newton iters (DVE only)
        mv = stats_all[:, s0:s0 + ns, :, 0]
        sv = stats_all[:, s0:s0 + ns, :, 1]
        msq = stat.tile([2, ns, CT], F32, tag="msq")
        nc.vector.tensor_mul(out=msq[:], in0=mv, in1=mv)
        nc.vector.tensor_sub(out=sv, in0=sv, in1=msq[:])
        vadd = stat.tile([2, ns, CT], F32, tag="vadd")
        nc.vector.tensor_scalar_add(out=vadd[:], in0=sv, scalar1=float(EPS))
        z = stat.tile([2, ns, CT], F32, tag="z")
        z_i = z[:].bitcast(I32)
        nc.vector.tensor_scalar(out=z_i, in0=vadd[:].bitcast(I32), scalar1=1,
                                scalar2=None, op0=ALU.arith_shift_right)
        mg = magic[:, 0:ns * CT].rearrange("p (s t) -> p s t", t=CT)
        nc.vector.scalar_tensor_tensor(out=z_i, in0=mg, scalar=0, in1=z_i,
                                       op0=ALU.bypass, op1=ALU.subtract)
        nt_ = stat.tile([2, ns, CT], F32, tag="nt")
        for _ in range(3):
            nc.vector.tensor_mul(out=nt_[:], in0=z[:], in1=z[:])
            nc.vector.tensor_mul(out=nt_[:], in0=nt_[:], in1=vadd[:])
            nc.vector.tensor_scalar(out=nt_[:], in0=nt_[:], scalar1=-0.5,
                                    scalar2=1.5, op0=ALU.mult, op1=ALU.add)
            nc.vector.tensor_mul(out=z[:], in0=z[:], in1=nt_[:])
        nc.vector.tensor_copy(out=sv, in_=z[:])
        ab = stat.tile([2, ns, CT, 2], F32, tag="ab")
        nc.vector.tensor_copy(out=ab[:, :, :, 0], in_=sv)
        nc.vector.scalar_tensor_tensor(out=ab[:, :, :, 1], in0=mv, scalar=-1.0,
                                       in1=sv, op0=ALU.mult, op1=ALU.mult)
        sb_ps = psum_sm.tile([P, ns * CT * 2], F32, tag="sm")
        nc.tensor.matmul(sb_ps[:], member_t[:],
                         ab[:].rearrange("p s t j -> p (s t j)"),
                         start=True, stop=True)
        sb = stat.tile([P, ns, CT, 2], F32, tag="sb")
        nc.vector.tensor_copy(
            out=sb[:], in_=sb_ps[:].rearrange("p (s t j) -> p s t j", t=CT, j=2))
        for i in range(ns):
            s = s0 + i
            for ct in range(CT):
                nc.vector.tensor_scalar_mul(out=sc[:, s, ct:ct + 1],
                                            in0=gamma_c[:, ct:ct + 1],
                                            scalar1=sb[:, i, ct, 0:1])
                nc.vector.scalar_tensor_tensor(out=sh[:, s, ct:ct + 1],
                                               in0=gamma_c[:, ct:ct + 1],
                                               scalar=sb[:, i, ct, 1:2],
                                               in1=beta_c[:, ct:ct + 1],
                                               op0=ALU.mult, op1=ALU.add)
            nc.vector.tensor_add(out=shp[:, s, :], in0=sh[:, s, :], in1=bp_c)

    # ======== phase 0: sample 0 stats ========
    x_dma(0, split=True)
    gn_stats(0)
    gn_affine(0, 1)

    # ======== main per-sample loop ========
    for s in range(SPC):
        if s + 2 < SPC:
            x_dma(s + 2)
        if s == 0 and SPC > 1:
            x_dma(1)

        # ---- xn: fp8 matmul copy + bf16 residual (bias pre-folded) ----
        xn_mm = xnpool.tile([P, CT, HW], MMDT, tag="xnmm")
        xn_res = xnpool.tile([P, CT, HW], BF16, tag="xnres")
        for ct in range(CT):
            nc.scalar.activation(out=xn_mm[:, ct, :], in_=xr[(s, ct)][:],
                                 func=AF.Identity,
                                 bias=sh[:, s, ct:ct + 1],
                                 scale=sc[:, s, ct:ct + 1])
            nc.gpsimd.tensor_scalar(out=xn_res[:, ct, :], in0=xr[(s, ct)][:],
                                    scalar1=sc[:, s, ct:ct + 1],
                                    scalar2=shp[:, s, ct:ct + 1],
                                    op0=ALU.mult, op1=ALU.add)

        # ---- u = A.T @ xn + cvec ----
        u = upool.tile([P, CT, HW], MMDT, tag="u")
        for dt in range(CT):
            ps = psum_big.tile([P, HW], F32, tag="big")
            for jc in range(NJC):
                mm_chain(ps[:, jc * FREE:(jc + 1) * FREE],
                         at_t, slice(dt * P, (dt + 1) * P),
                         xn_mm, slice(jc * FREE, (jc + 1) * FREE), CT)
            nc.vector.tensor_scalar_add(out=u[:, dt, :], in0=ps[:],
                                        scalar1=cvec_c[:, dt:dt + 1])

        # ---- vp[m, e] and eT[m, n] interleaved per token tile ----
        vp = vpool.tile([P, NT, FREE], MMDT, tag="vp")
        eT = epool.tile([P, NT, HW], MMDT, tag="e")
        for nt in range(NT):
            psv = psum_sm.tile([P, FREE], F32, tag="sm")
            mm_chain(psv[:], xn_mm, slice(nt * P, (nt + 1) * P),
                     btw_t, slice(0, C), CT)
            nc.vector.tensor_copy(out=vp[:, nt, :], in_=psv[:])
            pse = psum_big.tile([P, HW], F32, tag="big")
            for jc in range(NJC):
                mm_chain(pse[:, jc * FREE:(jc + 1) * FREE],
                         xn_mm, slice(nt * P, (nt + 1) * P),
                         u, slice(jc * FREE, (jc + 1) * FREE), CT)
            nc.scalar.activation(out=eT[:, nt, :], in_=pse[:], func=AF.Exp)

        # stats for the next sample hide under this sample's attention
        if s + 1 < SPC:
            gn_stats(s + 1)
            gn_affine(s + 1, 1)

        # ---- softmax denominator -> rb = (1/rowsum) broadcast ----
        rs_sb = rpool.tile([1, HW], BF16, tag="rs")
        for jc in range(NJC):
            rps = psum_sm.tile([1, FREE], F32, tag="sm")
            mm_chain(rps[:], ones2, slice(0, 1),
                     eT, slice(jc * FREE, (jc + 1) * FREE), NT)
            nc.vector.tensor_copy(out=rs_sb[:, jc * FREE:(jc + 1) * FREE],
                                  in_=rps[:])
        rb = rpool.tile([P, HW], F32, tag="rb")
        for jc in range(NJC):
            bps = psum_sm.tile([P, FREE], F32, tag="sm")
            nc.tensor.matmul(bps[:], ones_row[:],
                             rs_sb[:, jc * FREE:(jc + 1) * FREE],
                             start=True, stop=True)
            nc.vector.reciprocal_approx_fast(out=rb[:, jc * FREE:(jc + 1) * FREE],
                                             in_=bps[:])

        # ---- out = (vp.T @ eT) * rb + xn_res ----
        for et in range(CT):
            ps = psum_big.tile([P, HW], F32, tag="big")
            for jc in range(NJC):
                mm_chain(ps[:, jc * FREE:(jc + 1) * FREE],
                         vp, slice(et * P, (et + 1) * P),
                         eT, slice(jc * FREE, (jc + 1) * FREE), NT)
            t = respool.tile([P, HW], BF16, tag="t")
            nc.vector.tensor_mul(out=t[:], in0=ps[:], in1=rb[:])
            res = respool.tile([P, HW], BF16, tag="res")
            nc.gpsimd.tensor_add(out=res[:], in0=t[:], in1=xn_res[:, et, :])
            nc.sync.dma_start(out=aps["out"][s, et * P:(et + 1) * P, :], in_=res[:])


def build():
    nc = bacc.Bacc("TRN2", target_bir_lowering=False, debug=False)
    aps = _declare_io(nc)
    with tile.TileContext(nc) as tc:
        with ExitStack() as ctx:
            _build_tile_kernel(ctx, tc, aps)
    nc.compile()
    return nc


_cached_nc = None


def _get_nc():
    global _cached_nc
    if _cached_nc is None:
        _cached_nc = build()
    return _cached_nc


def _host_inputs(gamma, beta, Wq, bq, Wk, bk, Wv, bv, Wp, bp):
    import ml_dtypes
    mmnp = ml_dtypes.float8_e4m3 if QUANT == "fp8" else ml_dtypes.bfloat16
    f64 = lambda a: np.asarray(a, dtype=np.float64)
    wcast = lambda a: np.ascontiguousarray(a.astype(np.float32).astype(mmnp))

    a_s = (f64(Wq).T @ f64(Wk)) * SCALE          # [c, d']
    cvec = (f64(Wk).T @ f64(bq)) * SCALE         # [d']
    btw = (f64(Wp) @ f64(Wv)).T                  # [c, e]
    bp_eff = f64(bp) + f64(Wp) @ f64(bv)

    member_t = np.zeros((2, P), np.float32)
    member_t[0, :GSZ] = 1.0
    member_t[1, GSZ:] = 1.0

    smalls = np.zeros((P, 19), np.float32)
    for i, v in enumerate((cvec, bp_eff, gamma, beta)):
        smalls[:, i * CT:(i + 1) * CT] = (
            np.asarray(v, np.float32).reshape(CT, P).T)
    smalls[:GSZ, 16] = 1.0 / GSZ
    smalls[GSZ:, 17] = 1.0 / GSZ
    smalls[:, 18] = 1.0

    return {
        "at": wcast(a_s),
        "btw": wcast(btw),
        "smalls": smalls,
        "member_t": member_t,
        "ones2": np.ones((P, NT, 16), mmnp),
        "ones_row": np.ones((1, P), ml_dtypes.bfloat16),
    }


def run(inputs, trace=False, **kw):
    """Returns (out [B,C,H,W], BassKernelResults)."""
    import ml_dtypes
    nc = _get_nc()
    x = np.ascontiguousarray(
        np.asarray(inputs["x"], np.float32).reshape(B, C, HW)
        .astype(ml_dtypes.bfloat16))
    common = _host_inputs(**{k: v for k, v in inputs.items() if k != "x"})
    in_maps = [dict(common, x=x[c * SPC:(c + 1) * SPC]) for c in range(N_CORES)]
    res = run_bass_kernel_spmd(nc, in_maps, core_ids=list(range(N_CORES)),
                               trace=trace, **kw)
    out = np.concatenate([res.results[c]["out"] for c in range(N_CORES)], axis=0)
    return out.astype(np.float32).reshape(B, C, H, W), res


def kernel(**inputs):
    out, _ = run(inputs)
    return out


# revision 22
# speedup vs baseline: 1.3927x; 1.0367x over previous
"""AttentionBlock (GroupNorm + single-head self-attention + residual) on 8 TRN2 cores.

Data-parallel over batch: 32 samples -> 4 per core; weights replicated.

Algebraic folds (softmax over keys m kills terms constant in m):
  scores'[n,m] = (A^T h[n] + c)^T h[m],  A = Wq^T Wk * C^-0.5, c = Wk^T bq * C^-0.5
  -> ONE projection u = A^T xn + c replaces both Q and K.
  attn @ V @ Wp^T = attn @ (h (Wp Wv)^T) + Wp bv
  -> ONE projection vp = Btw^T xn (Btw = (Wp Wv)^T) replaces V and out-proj;
     Wp bv joins bp in the residual shift.

Per-core program (per sample, N=1024 tokens, C=512 channels):
  xn_mm [c, n] fp8   = x*scale + shift        (matmul operand)
  xn_res[c, n] bf16  = x*scale + (shift + bp_eff)   (residual, bias pre-folded)
  u   [d', n] fp8    = A-slices.T @ xn_mm + c       (lhsT = host A)
  vp  [m, e]  fp8    = xn-slices.T @ Btw            (keys on partitions)
  eT  [m, n]  fp8    = exp(xn-slices.T @ u)         (scale folded into A)
  rs  [1, n]         = ones2.T @ eT                 (DoubleRow ones matmul)
  rb  [128, n] f32   = 1/rs broadcast via K=1 bf16 matmul + reciprocal
  out [e, n] bf16    = (vp-slices.T @ eT) * rb + xn_res  -> DMA out

All matmuls fp8_e4m3 with MatmulPerfMode.DoubleRow (2 k-subtiles of 128
packed per instruction, 0.5 cycles/row = 2x bf16 rate). fp32 PSUM.
x is host-cast to bf16 (halves input DMA), out returned bf16 and upcast
on host. Emulated end-to-end rel err ~7e-3 vs 2e-2 budget.

GroupNorm stats are just-in-time: x[s+2] DMA issued at sample s's top,
stats+affine for s+1 run mid-sample-s (hidden under attention).
"""

import sys

if "/opt/trn_rl_repo" not in sys.path:
    sys.path.insert(0, "/opt/trn_rl_repo")

from contextlib import ExitStack

import numpy as np

import concourse.bass as bass
import concourse.tile as tile
from concourse import bacc, mybir
from concourse.bass_utils import run_bass_kernel_spmd

N_CORES = 8
B, C, H, W = 32, 512, 32, 32
HW = H * W            # tokens per sample (N)
SPC = B // N_CORES    # samples per core
G = 8                 # groups
GSZ = C // G          # channels per group (64)
EPS = 1e-5
P = 128               # partitions
CT = C // P           # channel subtiles (4)
NT = HW // P          # token subtiles (8)
SCALE = C ** -0.5
FREE = 512            # PSUM one-bank free width (fp32)
NJC = HW // FREE      # 2

F32 = mybir.dt.float32
BF16 = mybir.dt.bfloat16
FP8 = mybir.dt.float8e4
AF = mybir.ActivationFunctionType
ALU = mybir.AluOpType
DR = mybir.MatmulPerfMode.DoubleRow

QUANT = "fp8"         # "fp8" (DoubleRow) or "bf16" (plain matmuls)
MMDT = FP8 if QUANT == "fp8" else BF16


def _declare_io(nc):
    def inp(name, shape, dt):
        return nc.dram_tensor(name, list(shape), dt, kind="ExternalInput").ap()

    aps = {
        "x": inp("x", (SPC, C, HW), BF16),
        "at": inp("at", (C, C), MMDT),           # A[c, d'] (scale folded in)
        "btw": inp("btw", (C, C), MMDT),         # Btw[c, e] = (Wp Wv)^T
        "smalls": inp("smalls", (P, 19), F32),
        "member_t": inp("member_t", (2, P), F32),
        "ones2": inp("ones2", (P, NT, 16), MMDT),  # 16-wide: dual-fp8 ldweights needs 16B-aligned k-pair stride
        "ones_row": inp("ones_row", (1, P), BF16),
        "out": nc.dram_tensor("out", [SPC, C, HW], BF16, kind="ExternalOutput").ap(),
    }
    return aps


def _build_tile_kernel(ctx: ExitStack, tc: tile.TileContext, aps):
    nc = tc.nc

    singles = ctx.enter_context(tc.tile_pool(name="singles", bufs=1))
    wpool = ctx.enter_context(tc.tile_pool(name="wpool", bufs=1))
    xall = ctx.enter_context(tc.tile_pool(name="xall", bufs=1))
    xnpool = ctx.enter_context(tc.tile_pool(name="xnpool", bufs=2))
    upool = ctx.enter_context(tc.tile_pool(name="upool", bufs=2))
    vpool = ctx.enter_context(tc.tile_pool(name="vpool", bufs=2))
    epool = ctx.enter_context(tc.tile_pool(name="epool", bufs=2))
    rpool = ctx.enter_context(tc.tile_pool(name="rpool", bufs=2))
    respool = ctx.enter_context(tc.tile_pool(name="respool", bufs=3))
    stat = ctx.enter_context(tc.tile_pool(name="stat", bufs=2))
    sall = ctx.enter_context(tc.tile_pool(name="sall", bufs=1))

    psum_big = ctx.enter_context(tc.tile_pool(name="psum_big", bufs=3, space="PSUM"))
    psum_sm = ctx.enter_context(tc.tile_pool(name="psum_sm", bufs=2, space="PSUM"))

    # ---- small constants ----
    smalls = singles.tile([P, 19], F32, tag="smalls")
    nc.sync.dma_start(out=smalls[:], in_=aps["smalls"][:])
    cvec_c = smalls[:, 0:CT]
    bp_c = smalls[:, CT:2 * CT]
    gamma_c = smalls[:, 2 * CT:3 * CT]
    beta_c = smalls[:, 3 * CT:4 * CT]
    member = smalls[:, 16:18]
    member_t = singles.tile([2, P], F32, tag="member_t")
    nc.sync.dma_start(out=member_t[:], in_=aps["member_t"][:])
    ones2 = singles.tile([P, NT, 16], MMDT, tag="ones2")
    nc.sync.dma_start(out=ones2[:], in_=aps["ones2"][:])
    ones_row = singles.tile([1, P], BF16, tag="ones_row")
    nc.sync.dma_start(out=ones_row[:], in_=aps["ones_row"][:])

    I32 = mybir.dt.int32
    magic = singles.tile([2, SPC * CT], I32, tag="magic")
    nc.vector.memset(magic[:], 0x5F3759DF)

    # prime the ACT Exp spline table before any real dependency needs it
    warm = singles.tile([1, 1], F32, tag="warm")
    nc.vector.memset(warm[:], 1.0)
    nc.scalar.activation(out=warm[:], in_=warm[:], func=AF.Exp)

    # ---- weights resident in SBUF: [128, CT(k-subtile), C] ----
    at_t = wpool.tile([P, CT, C], MMDT, tag="at")
    btw_t = wpool.tile([P, CT, C], MMDT, tag="btw")
    wdmas = []
    for ct in range(CT):
        wdmas.append(nc.gpsimd.dma_start(out=at_t[:, ct, :],
                                         in_=aps["at"][ct * P:(ct + 1) * P, :]))
        wdmas.append(nc.gpsimd.dma_start(out=btw_t[:, ct, :],
                                         in_=aps["btw"][ct * P:(ct + 1) * P, :]))

    # ---- DoubleRow matmul chain helper ----
    # lhsT, rhs: [128, nk, *] SBUF tiles; msl/nsl slices of the last dim.
    def mm_chain(ps, lhsT, msl, rhs, nsl, nk):
        if QUANT == "fp8":
            for i, k in enumerate(range(0, nk, 2)):
                nc.tensor.matmul(ps, lhsT[:, k:k + 2, msl], rhs[:, k:k + 2, nsl],
                                 start=(i == 0), stop=(k + 2 >= nk), perf_mode=DR)
        else:
            for k in range(nk):
                nc.tensor.matmul(ps, lhsT[:, k, msl], rhs[:, k, nsl],
                                 start=(k == 0), stop=(k == nk - 1))

    # ======== GroupNorm statistics (per-sample, just-in-time) ========
    # stats_all[j, s, t, k]: group-local j, sample s, channel-tile t,
    # k=0 mean / k=1 E[x^2]; sc/sh/shp: per-channel scale/shift columns.
    stats_all = sall.tile([2, SPC, CT, 2], F32, tag="stats_all")
    sc = sall.tile([P, SPC, CT], F32, tag="sc")
    sh = sall.tile([P, SPC, CT], F32, tag="sh")
    shp = sall.tile([P, SPC, CT], F32, tag="shp")

    xr = {}  # (s, ct) -> resident x tile (bf16)

    def x_dma(s, split=False):
        for ct in range(CT):
            xt = xall.tile([P, HW], BF16, tag=f"x{s % 3}_{ct}")
            if split:
                nc.sync.dma_start(out=xt[:, 0:512],
                                  in_=aps["x"][s, ct * P:(ct + 1) * P, 0:512])
                nc.sync.dma_start(out=xt[:, 512:1024],
                                  in_=aps["x"][s, ct * P:(ct + 1) * P, 512:1024])
            else:
                nc.sync.dma_start(out=xt[:], in_=aps["x"][s, ct * P:(ct + 1) * P, :])
            xr[(s, ct)] = xt

    def gn_stats(s):
        partials = stat.tile([P, CT, 2], F32, tag="partials")
        for ct in range(CT):
            xt = xr[(s, ct)]
            st6 = stat.tile([P, 2, 6], F32, tag="st6")
            nc.vector.bn_stats(out=st6[:, 0, :], in_=xt[:, 0:512])
            nc.vector.bn_stats(out=st6[:, 1, :], in_=xt[:, 512:1024])
            nc.vector.bn_aggr(out=partials[:, ct, :], in_=st6[:])
            nc.vector.scalar_tensor_tensor(
                out=partials[:, ct, 1:2], in0=partials[:, ct, 0:1],
                scalar=partials[:, ct, 0:1], in1=partials[:, ct, 1:2],
                op0=ALU.mult, op1=ALU.add)
        ps = psum_sm.tile([2, CT * 2], F32, tag="sm")
        nc.tensor.matmul(ps[:], member[:],
                         partials[:].rearrange("p t j -> p (t j)"),
                         start=True, stop=True)
        nc.vector.tensor_copy(out=stats_all[:, s, :, :],
                              in_=ps[:].rearrange("p (t j) -> p t j", j=2))

    def gn_affine(s0, ns):
        # rstd = rsqrt(var+eps) via bit-trick + 3 Newton iters (DVE only)
        mv = stats_all[:, s0:s0 + ns, :, 0]
        sv = stats_all[:, s0:s0 + ns, :, 1]
        msq = stat.tile([2, ns, CT], F32, tag="msq")
        nc.vector.tensor_mul(out=msq[:], in0=mv, in1=mv)
        nc.vector.tensor_sub(out=sv, in0=sv, in1=msq[:])
        vadd = stat.tile([2, ns, CT], F32, tag="vadd")
        nc.vector.tensor_scalar_add(out=vadd[:], in0=sv, scalar1=float(EPS))
        z = stat.tile([2, ns, CT], F32, tag="z")
        z_i = z[:].bitcast(I32)
        nc.vector.tensor_scalar(out=z_i, in0=vadd[:].bitcast(I32), scalar1=1,
                                scalar2=None, op0=ALU.arith_shift_right)
        mg = magic[:, 0:ns * CT].rearrange("p (s t) -> p s t", t=CT)
        nc.vector.scalar_tensor_tensor(out=z_i, in0=mg, scalar=0, in1=z_i,
                                       op0=ALU.bypass, op1=ALU.subtract)
        nt_ = stat.tile([2, ns, CT], F32, tag="nt")
        for _ in range(3):
            nc.vector.tensor_mul(out=nt_[:], in0=z[:], in1=z[:])
            nc.vector.tensor_mul(out=nt_[:], in0=nt_[:], in1=vadd[:])
            nc.vector.tensor_scalar(out=nt_[:], in0=nt_[:], scalar1=-0.5,
                                    scalar2=1.5, op0=ALU.mult, op1=ALU.add)
            nc.vector.tensor_mul(out=z[:], in0=z[:], in1=nt_[:])
        nc.vector.tensor_copy(out=sv, in_=z[:])
        ab = stat.tile([2, ns, CT, 2], F32, tag="ab")
        nc.vector.tensor_copy(out=ab[:, :, :, 0], in_=sv)
        nc.vector.scalar_tensor_tensor(out=ab[:, :, :, 1], in0=mv, scalar=-1.0,
                                       in1=sv, op0=ALU.mult, op1=ALU.mult)
        sb_ps = psum_sm.tile([P, ns * CT * 2], F32, tag="sm")
        nc.tensor.matmul(sb_ps[:], member_t[:],
                         ab[:].rearrange("p s t j -> p (s t j)"),
                         start=True, stop=True)
        sb = stat.tile([P, ns, CT, 2], F32, tag="sb")
        nc.vector.tensor_copy(
            out=sb[:], in_=sb_ps[:].rearrange("p (s t j) -> p s t j", t=CT, j=2))
        for i in range(ns):
            s = s0 + i
            for ct in range(CT):
                nc.vector.tensor_scalar_mul(out=sc[:, s, ct:ct + 1],
                                            in0=gamma_c[:, ct:ct + 1],
                                            scalar1=sb[:, i, ct, 0:1])
                nc.vector.scalar_tensor_tensor(out=sh[:, s, ct:ct + 1],
                                               in0=gamma_c[:, ct:ct + 1],
                                               scalar=sb[:, i, ct, 1:2],
                                               in1=beta_c[:, ct:ct + 1],
                                               op0=ALU.mult, op1=ALU.add)
            nc.vector.tensor_add(out=shp[:, s, :], in0=sh[:, s, :], in1=bp_c)

    # ======== phase 0: sample 0 stats ========
    x_dma(0, split=True)
    gn_stats(0)
    gn_affine(0, 1)

    # ======== main per-sample loop ========
    for s in range(SPC):
        if s + 2 < SPC:
            x_dma(s + 2)
        if s == 0 and SPC > 1:
            x_dma(1)

        # ---- xn: fp8 matmul copy + bf16 residual (bias pre-folded) ----
        xn_mm = xnpool.tile([P, CT, HW], MMDT, tag="xnmm")
        xn_res = xnpool.tile([P, CT, HW], BF16, tag="xnres")
        for ct in range(CT):
            nc.scalar.activation(out=xn_mm[:, ct, :], in_=xr[(s, ct)][:],
                                 func=AF.Identity,
                                 bias=sh[:, s, ct:ct + 1],
                                 scale=sc[:, s, ct:ct + 1])
            nc.gpsimd.tensor_scalar(out=xn_res[:, ct, :], in0=xr[(s, ct)][:],
                                    scalar1=sc[:, s, ct:ct + 1],
                                    scalar2=shp[:, s, ct:ct + 1],
                                    op0=ALU.mult, op1=ALU.add)

        # ---- u = A.T @ xn + cvec ----
        u = upool.tile([P, CT, HW], MMDT, tag="u")
        for dt in range(CT):
            ps = psum_big.tile([P, HW], F32, tag="big")
            for jc in range(NJC):
                mm_chain(ps[:, jc * FREE:(jc + 1) * FREE],
                         at_t, slice(dt * P, (dt + 1) * P),
                         xn_mm, slice(jc * FREE, (jc + 1) * FREE), CT)
            nc.scalar.activation(out=u[:, dt, :], in_=ps[:], func=AF.Identity,
                                 bias=cvec_c[:, dt:dt + 1], scale=1.0)

        # ---- vp[m, e] and eT[m, n] interleaved per token tile ----
        vp = vpool.tile([P, NT, FREE], MMDT, tag="vp")
        eT = epool.tile([P, NT, HW], MMDT, tag="e")
        for nt in range(NT):
            psv = psum_sm.tile([P, FREE], F32, tag="sm")
            mm_chain(psv[:], xn_mm, slice(nt * P, (nt + 1) * P),
                     btw_t, slice(0, C), CT)
            nc.vector.tensor_copy(out=vp[:, nt, :], in_=psv[:])
            pse = psum_big.tile([P, HW], F32, tag="big")
            for jc in range(NJC):
                mm_chain(pse[:, jc * FREE:(jc + 1) * FREE],
                         xn_mm, slice(nt * P, (nt + 1) * P),
                         u, slice(jc * FREE, (jc + 1) * FREE), CT)
            nc.scalar.activation(out=eT[:, nt, :], in_=pse[:], func=AF.Exp)

        # stats for the next sample hide under this sample's attention
        if s + 1 < SPC:
            gn_stats(s + 1)
            gn_affine(s + 1, 1)

        # ---- softmax denominator -> rb = (1/rowsum) broadcast ----
        rs_sb = rpool.tile([1, HW], BF16, tag="rs")
        for jc in range(NJC):
            rps = psum_sm.tile([1, FREE], F32, tag="sm")
            mm_chain(rps[:], ones2, slice(0, 1),
                     eT, slice(jc * FREE, (jc + 1) * FREE), NT)
            nc.vector.tensor_copy(out=rs_sb[:, jc * FREE:(jc + 1) * FREE],
                                  in_=rps[:])
        rb = rpool.tile([P, HW], F32, tag="rb")
        for jc in range(NJC):
            bps = psum_sm.tile([P, FREE], F32, tag="sm")
            nc.tensor.matmul(bps[:], ones_row[:],
                             rs_sb[:, jc * FREE:(jc + 1) * FREE],
                             start=True, stop=True)
            nc.vector.reciprocal_approx_fast(out=rb[:, jc * FREE:(jc + 1) * FREE],
                                             in_=bps[:])

        # ---- out = (vp.T @ eT) * rb + xn_res ----
        for et in range(CT):
            ps = psum_big.tile([P, HW], F32, tag="big")
            for jc in range(NJC):
                mm_chain(ps[:, jc * FREE:(jc + 1) * FREE],
                         vp, slice(et * P, (et + 1) * P),
                         eT, slice(jc * FREE, (jc + 1) * FREE), NT)
            t = respool.tile([P, HW], BF16, tag="t")
            nc.vector.tensor_mul(out=t[:], in0=ps[:], in1=rb[:])
            res = respool.tile([P, HW], BF16, tag="res")
            nc.gpsimd.tensor_add(out=res[:], in0=t[:], in1=xn_res[:, et, :])
            nc.sync.dma_start(out=aps["out"][s, et * P:(et + 1) * P, :], in_=res[:])


def build():
    nc = bacc.Bacc("TRN2", target_bir_lowering=False, debug=False)
    aps = _declare_io(nc)
    with tile.TileContext(nc) as tc:
        with ExitStack() as ctx:
            _build_tile_kernel(ctx, tc, aps)
    nc.compile()
    return nc


_cached_nc = None


def _get_nc():
    global _cached_nc
    if _cached_nc is None:
        _cached_nc = build()
    return _cached_nc


def _host_inputs(gamma, beta, Wq, bq, Wk, bk, Wv, bv, Wp, bp):
    import ml_dtypes
    mmnp = ml_dtypes.float8_e4m3 if QUANT == "fp8" else ml_dtypes.bfloat16
    f64 = lambda a: np.asarray(a, dtype=np.float64)
    wcast = lambda a: np.ascontiguousarray(a.astype(np.float32).astype(mmnp))

    a_s = (f64(Wq).T @ f64(Wk)) * SCALE          # [c, d']
    cvec = (f64(Wk).T @ f64(bq)) * SCALE         # [d']
    btw = (f64(Wp) @ f64(Wv)).T                  # [c, e]
    bp_eff = f64(bp) + f64(Wp) @ f64(bv)

    member_t = np.zeros((2, P), np.float32)
    member_t[0, :GSZ] = 1.0
    member_t[1, GSZ:] = 1.0

    smalls = np.zeros((P, 19), np.float32)
    for i, v in enumerate((cvec, bp_eff, gamma, beta)):
        smalls[:, i * CT:(i + 1) * CT] = (
            np.asarray(v, np.float32).reshape(CT, P).T)
    smalls[:GSZ, 16] = 1.0 / GSZ
    smalls[GSZ:, 17] = 1.0 / GSZ
    smalls[:, 18] = 1.0

    return {
        "at": wcast(a_s),
        "btw": wcast(btw),
        "smalls": smalls,
        "member_t": member_t,
        "ones2": np.ones((P, NT, 16), mmnp),
        "ones_row": np.ones((1, P), ml_dtypes.bfloat16),
    }


def run(inputs, trace=False, **kw):
    """Returns (out [B,C,H,W], BassKernelResults)."""
    import ml_dtypes
    nc = _get_nc()
    x = np.ascontiguousarray(
        np.asarray(inputs["x"], np.float32).reshape(B, C, HW)
        .astype(ml_dtypes.bfloat16))
    common = _host_inputs(**{k: v for k, v in inputs.items() if k != "x"})
    in_maps = [dict(common, x=x[c * SPC:(c + 1) * SPC]) for c in range(N_CORES)]
    res = run_bass_kernel_spmd(nc, in_maps, core_ids=list(range(N_CORES)),
                               trace=trace, **kw)
    out = np.concatenate([res.results[c]["out"] for c in range(N_CORES)], axis=0)
    return out.astype(np.float32).reshape(B, C, H, W), res


def kernel(**inputs):
    out, _ = run(inputs)
    return out
